# revision 1
# baseline (speedup 1.0000x reference)
"""Trainium2 Bass kernel for nn_AugmentedConv (conv branch + conv-attention branch).

Full-input contract: kernel(**inputs) takes the complete unsharded inputs and
returns the full (8, 512, 2048) output. Internally: data-parallel over batch
across 8 NeuronCores; each core runs the whole module for one batch element.

Hardcoded problem shapes: B=8, C=256, W=2048, DK=DV=256, NH=8, KS=3, pad=1.
"""

import numpy as np

import concourse.bacc as bacc
import concourse.mybir as mybir
import concourse.tile as tile
from concourse import bass_utils
from concourse.masks import make_identity

F32 = mybir.dt.float32
F16 = mybir.dt.float16
ESHIFT = -4.0   # exp(x + ESHIFT): keeps fp16 exp in range; cancels in softmax ratio

C = 256          # input channels
W = 2048         # sequence length
OC = 1024        # combined conv output channels: [conv_out 256 | q 256 | k 256 | v 256]
NH = 8
DKH = 32         # head dim (dk and dv per head)
QSCALE = float(DKH) ** -0.5
NCT = C // 128   # input-channel tiles (2)
NWT = W // 512   # 512-wide w tiles (4)
NMT = W // 128   # 128-wide w tiles (16)


def build_nc():
    nc = bacc.Bacc("TRN2", target_bir_lowering=False, debug=False)

    x_d = nc.dram_tensor("x", [C, W], F16, kind="ExternalInput")
    wt_d = nc.dram_tensor("wt", [3, C, OC], F16, kind="ExternalInput")     # wt[k,c,oc]
    ball_d = nc.dram_tensor("b_all", [OC], F32, kind="ExternalInput")      # q part pre-scaled
    watt_d = nc.dram_tensor("watT", [C, C], F16, kind="ExternalInput")     # w_attn.T (c,o)
    bat_d = nc.dram_tensor("bat", [C], F32, kind="ExternalInput")
    out_d = nc.dram_tensor("out", [2 * C, W], F32, kind="ExternalOutput")
    # attention output staged in [h, w, d] order; the module's faithful
    # (NH,W,dvh)->(256,W) reshape is then a contiguous view of this buffer.
    ahwd_d = nc.dram_tensor("attn_hwd", [NH, W, DKH], F16)

    with tile.TileContext(nc) as tc:
        import contextlib
        with contextlib.ExitStack() as ctx:
            singles = ctx.enter_context(tc.tile_pool(name="singles", bufs=1))
            xp = ctx.enter_context(tc.tile_pool(name="xp", bufs=NCT))
            wtp = ctx.enter_context(tc.tile_pool(name="wtp", bufs=3 * NCT))
            qkp = ctx.enter_context(tc.tile_pool(name="qkp", bufs=4))
            vtp = ctx.enter_context(tc.tile_pool(name="vtp", bufs=NMT))
            ep = ctx.enter_context(tc.tile_pool(name="ep", bufs=8))
            stage = ctx.enter_context(tc.tile_pool(name="stage", bufs=4))
            norm = ctx.enter_context(tc.tile_pool(name="norm", bufs=8))
            qpp = ctx.enter_context(tc.tile_pool(name="qpp", bufs=2))

            # ---- constants -------------------------------------------------
            ident = singles.tile([128, 128], F32)
            make_identity(nc, ident[:])
            b_sb = singles.tile([128, 8], F32)  # conv biases, [ch-in-tile, oc-tile]
            nc.gpsimd.dma_start(out=b_sb, in_=ball_d.ap().rearrange("(t p) -> p t", p=128))
            bat_sb = singles.tile([128, 2], F32)
            nc.gpsimd.dma_start(out=bat_sb, in_=bat_d.ap().rearrange("(t p) -> p t", p=128))
            bv_sb = singles.tile([128, C], F32)  # v bias replicated across partitions
            nc.gpsimd.dma_start(
                out=bv_sb, in_=ball_d.ap()[3 * C:4 * C].partition_broadcast(128))
            zero1 = singles.tile([128, 1], F32)
            nc.vector.memset(zero1[:], 0.0)
            eshift_sb = singles.tile([128, 1], F32)
            nc.vector.memset(eshift_sb[:], ESHIFT)
            ones8 = singles.tile([128, 8, 1], F32)
            nc.vector.memset(ones8[:], 1.0)
            wup = singles.tile([128, 512], F16)  # PE warm-up fodder
            nc.vector.memset(wup[:], 0.0)

            # ---- load x (zero-padded by one column each side) and weights --
            x_sb = []
            for ct in range(NCT):
                t = xp.tile([128, W + 2], F16, tag="x", name=f"x{ct}")
                nc.vector.tensor_copy(t[:, 0:1], zero1[:])
                nc.vector.tensor_copy(t[:, W + 1:W + 2], zero1[:])
                nc.gpsimd.dma_start(out=t[:, 1:W + 1], in_=x_d.ap()[ct * 128:(ct + 1) * 128, :])
                x_sb.append(t)
            wt_sb = {}
            wtv_sb = {}
            for kk in range(3):
                for ct in range(NCT):
                    t = wtp.tile([128, OC], F16, tag="wt", name=f"wt{kk}_{ct}")
                    nc.gpsimd.dma_start(out=t, in_=wt_d.ap()[kk, ct * 128:(ct + 1) * 128, :])
                    wt_sb[kk, ct] = t
                    wtv_sb[kk, ct] = t[:, 3 * C:4 * C]
            watt_sb = []
            for ct in range(NCT):
                t = qkp.tile([128, C], F16, tag="watt", name=f"watt{ct}")
                nc.gpsimd.dma_start(out=t, in_=watt_d.ap()[ct * 128:(ct + 1) * 128, :])
                watt_sb.append(t)

            # ---- stage 1: q/k convs + vT conv ([ch,W] and [w,ch] layouts) --
            q_sb, k_sb = [], []
            for qt in range(2):
                q_sb.append(qkp.tile([128, W], F16, tag="qk", name=f"q{qt}"))
                k_sb.append(qkp.tile([128, W], F16, tag="qk", name=f"k{qt}"))

            with tc.tile_pool(name="cps", bufs=4, space="PSUM") as cps:
                # PE warm-up burst: gets HAM to 8/8 while the input DMAs run
                wps = cps.tile([128, 512], F32, tag="cps", name="wps")
                for _ in range(26):
                    nc.tensor.matmul(wps[:], wup[:, 0:128], wup[:], start=True, stop=True)

                # vT conv: [w, vch] layout (x slice is the stationary operand),
                # v bias added here; col 32 of each 33-wide head block stays 1.0
                # (softmax-denominator ones column), cols >=264 zero pad.
                vt_sb = []
                for m in range(NMT):
                    vt = vtp.tile([128, NH * 33 + 96], F16, tag="vt", name=f"vt{m}")
                    nc.vector.tensor_copy(
                        vt[:, 0:NH * 33].rearrange("p (h e) -> p h e", e=33)[:, :, 32:33],
                        ones8[:])
                    nc.vector.memset(vt[:, NH * 33:], 0.0)
                    ps = cps.tile([128, C], F32, tag="vps")
                    for ct in range(NCT):
                        for kk in range(3):
                            nc.tensor.matmul(
                                ps[:],
                                x_sb[ct][:, m * 128 + kk:m * 128 + kk + 128],
                                wtv_sb[kk, ct],
                                start=(ct == 0 and kk == 0),
                                stop=(ct == NCT - 1 and kk == 2),
                            )
                    nc.vector.tensor_add(
                        vt[:, 0:NH * 33].rearrange("p (h e) -> p h e", e=33)[:, :, 0:32],
                        ps[:].rearrange("p (h d) -> p h d", d=32),
                        bv_sb[:].rearrange("p (h d) -> p h d", d=32),
                    )
                    vt_sb.append(vt)

                for t in (2, 4):  # oc-tiles: q0, k0 (tile1 deferred)
                    for n in range(NWT):
                        ps = cps.tile([128, 512], F32, tag="cps")
                        for ct in range(NCT):
                            for kk in range(3):
                                nc.tensor.matmul(
                                    ps[:],
                                    wt_sb[kk, ct][:, t * 128:(t + 1) * 128],
                                    x_sb[ct][:, n * 512 + kk:n * 512 + kk + 512],
                                    start=(ct == 0 and kk == 0),
                                    stop=(ct == NCT - 1 and kk == 2),
                                )
                        ns = slice(n * 512, (n + 1) * 512)
                        if t < 4:        # q (scale folded; bias pre-scaled on host)
                            nc.vector.tensor_scalar(
                                out=q_sb[t - 2][:, ns], in0=ps[:],
                                scalar1=QSCALE, scalar2=b_sb[:, t:t + 1],
                                op0=mybir.AluOpType.mult, op1=mybir.AluOpType.add)
                        else:            # k
                            nc.vector.tensor_scalar_add(k_sb[t - 4][:, ns], ps[:], b_sb[:, t:t + 1])

            # ---- stage 2: attention, one head at a time, every matmul in the
            # same (128,128) PE tile mode so the HAM clock stays warm:
            #  - logits contract over all 128 partitions against a zero-padded q
            #  - attn lhsT is a 128-col window of vt (rows >=33 are ignored)
            # Deferred conv units (conv_out rows, q/k tile 1) are interleaved
            # under the ScalarE(exp) shadow, which is the kernel bottleneck.
            def build_qpad(h):
                qt = h // 4
                s = 32 * (h % 4)
                qpad = qpp.tile([128, W], F16, tag="qpad", name=f"qpad{h}")
                nc.vector.memset(qpad[:], 0.0)
                nc.vector.tensor_copy(qpad[s:s + 32, :], q_sb[qt][s:s + 32, :])
                return qpad

            with tc.tile_pool(name="lg", bufs=2, space="PSUM") as lg, \
                 tc.tile_pool(name="aps", bufs=2, space="PSUM") as aps, \
                 tc.tile_pool(name="tps", bufs=2, space="PSUM") as tps:

                def conv_unit(t, n):
                    ps = tps.tile([128, 512], F32, tag="tps", name=f"cops{t}_{n}")
                    for ct in range(NCT):
                        for kk in range(3):
                            nc.tensor.matmul(
                                ps[:],
                                wt_sb[kk, ct][:, t * 128:(t + 1) * 128],
                                x_sb[ct][:, n * 512 + kk:n * 512 + kk + 512],
                                start=(ct == 0 and kk == 0),
                                stop=(ct == NCT - 1 and kk == 2),
                            )
                    ns = slice(n * 512, (n + 1) * 512)
                    if t < 2:      # conv_out -> DRAM
                        co = stage.tile([128, 512], F32, tag="co")
                        nc.vector.tensor_scalar_add(co[:], ps[:], b_sb[:, t:t + 1])
                        nc.sync.dma_start(out=out_d.ap()[t * 128:(t + 1) * 128, ns], in_=co[:])
                    elif t == 3:   # q tile 1
                        nc.vector.tensor_scalar(
                            out=q_sb[1][:, ns], in0=ps[:],
                            scalar1=QSCALE, scalar2=b_sb[:, t:t + 1],
                            op0=mybir.AluOpType.mult, op1=mybir.AluOpType.add)
                    else:          # k tile 1
                        nc.vector.tensor_scalar_add(k_sb[1][:, ns], ps[:], b_sb[:, t:t + 1])

                qpad = build_qpad(0)
                # resident reshape source for stage 3: ar[c=(h,g), w2=(r,d)],
                # streamed back per finished head from the [h,w,d] DRAM staging
                ar_sb = [qkp.tile([128, W], F16, tag="ar", name=f"ar{i}") for i in range(2)]

                ar = ahwd_d.ap().rearrange("h (g x) d -> (h g) (x d)", g=32)  # [256, 2048]
                pending = []  # normalize work deferred by one half-iteration

                def flush_pending():
                    done_heads = {hh for hh, n, _ in pending if n == 3}
                    for hh, n, a_sb in pending:
                        for j in range(4):
                            js = slice(j * 128, (j + 1) * 128)
                            t_ps = tps.tile([128, 33], F32, tag="tps",
                                            name=f"tp{hh}_{n}_{j}")
                            nc.tensor.transpose(t_ps[:, 0:33], a_sb[:, js], ident[0:33, 0:33])
                            r_sb = norm.tile([128, 1], F32, tag="r", name=f"r{hh}_{n}_{j}")
                            nc.vector.reciprocal(r_sb[:], t_ps[:, 32:33])
                            z_sb = norm.tile([128, 32], F16, tag="z", name=f"z{hh}_{n}_{j}")
                            nc.vector.tensor_scalar_mul(z_sb[:], t_ps[:, 0:32], r_sb[:])
                            ws = slice(n * 512 + j * 128, n * 512 + j * 128 + 128)
                            nc.sync.dma_start(out=ahwd_d.ap()[hh, ws, :], in_=z_sb[:])
                    pending.clear()
                    for hh in sorted(done_heads):
                        # head fully staged out -> stream its reshape rows in
                        rr = slice((hh % 4) * 32, (hh % 4) * 32 + 32)
                        nc.sync.dma_start(out=ar_sb[hh // 4][rr, :],
                                          in_=ar[hh * 32:(hh + 1) * 32, :])

                for h in range(NH):
                    qt = h // 4
                    cur_qpad = qpad
                    for half in range(2):
                        # deferred conv units, hidden under the exp shadow.
                        # q tile 1 first: build_qpad(4) (emitted at h==3, half==1)
                        # reads it, and program order is semantic.
                        unit = 2 * h + half
                        if unit < 4:
                            conv_unit(3, unit)      # q tile 1
                        elif unit < 8:
                            conv_unit(5, unit - 4)  # k tile 1
                        elif unit < 16:
                            conv_unit((unit - 8) // 4, (unit - 8) % 4)  # conv_out
                        if half == 1 and h + 1 < NH:
                            qpad = build_qpad(h + 1)
                        acc = [aps.tile([128, 512], F32, tag="aps",
                                        name=f"acc{h}_{half}_{n2}") for n2 in range(2)]
                        for m in range(16):
                            ms = slice(m * 128, (m + 1) * 128)
                            lg_t = lg.tile([128, 1024], F32, tag="lg", name=f"lg{h}_{half}_{m}")
                            for n2 in range(2):
                                qs = slice(half * 1024 + n2 * 512, half * 1024 + n2 * 512 + 512)
                                nc.tensor.matmul(
                                    lg_t[:, n2 * 512:(n2 + 1) * 512],
                                    k_sb[qt][:, ms], cur_qpad[:, qs], start=True, stop=True)
                            e = ep.tile([128, 1024], F16, tag="e", name=f"e{h}_{half}_{m}")
                            nc.scalar.activation(e[:], lg_t[:],
                                                 mybir.ActivationFunctionType.Exp,
                                                 bias=eshift_sb[:])
                            for n2 in range(2):
                                nc.tensor.matmul(
                                    acc[n2][:], vt_sb[m][:, h * 33:h * 33 + 128],
                                    e[:, n2 * 512:(n2 + 1) * 512],
                                    start=(m == 0), stop=(m == 15))

                        # free the accumulators now; defer transposes one half
                        new_pending = []
                        for n2 in range(2):
                            n = 2 * half + n2
                            a_sb = norm.tile([33, 512], F32, tag="asb", name=f"a{h}_{half}_{n2}")
                            nc.vector.tensor_copy(a_sb[:], acc[n2][0:33, :])
                            new_pending.append((h, n, a_sb))
                        flush_pending()
                        pending = new_pending

                flush_pending()

                # ---- stage 3: 1x1 conv over the (faithful-reshape) view ----
                wk = lg.tile([128, 512], F32, tag="lg", name="warmkeep")
                for _ in range(12):
                    nc.tensor.matmul(wk[:], wup[:, 0:128], wup[:], start=True, stop=True)
                for t2 in range(2):
                    for n in range(NWT):
                        ps = lg.tile([128, 512], F32, tag="lg", name=f"fin{t2}_{n}")
                        for ct in range(NCT):
                            nc.tensor.matmul(
                                ps[:], watt_sb[ct][:, t2 * 128:(t2 + 1) * 128],
                                ar_sb[ct][:, n * 512:(n + 1) * 512],
                                start=(ct == 0), stop=(ct == NCT - 1))
                        fo = stage.tile([128, 512], F32, tag="fo")
                        nc.vector.tensor_scalar_add(fo[:], ps[:], bat_sb[:, t2:t2 + 1])
                        nc.sync.dma_start(
                            out=out_d.ap()[C + t2 * 128:C + (t2 + 1) * 128,
                                           n * 512:(n + 1) * 512],
                            in_=fo[:])

    nc.compile()
    return nc


_NC_CACHE = []


def _get_nc():
    if not _NC_CACHE:
        _NC_CACHE.append(build_nc())
    return _NC_CACHE[0]


def _prep_in_maps(x, w_conv, b_conv, w_qkv, b_qkv, w_attn, b_attn):
    x = np.asarray(x, np.float16)
    wt = np.ascontiguousarray(
        np.concatenate([np.asarray(w_conv, np.float32), np.asarray(w_qkv, np.float32)], 0)
        .transpose(2, 1, 0).astype(np.float16))                # [3, c, oc]
    b_all = np.concatenate([np.asarray(b_conv, np.float32),
                            np.asarray(b_qkv, np.float32)]).copy()
    b_all[C:2 * C] *= QSCALE                                   # q bias pre-scaled
    watt = np.ascontiguousarray(np.asarray(w_attn, np.float32).T.astype(np.float16))
    bat = np.ascontiguousarray(np.asarray(b_attn, np.float32))
    return [
        {"x": np.ascontiguousarray(x[b]), "wt": wt, "b_all": b_all,
         "watT": watt, "bat": bat}
        for b in range(x.shape[0])
    ]


def run(trace=False, **inputs):
    nc = _get_nc()
    in_maps = _prep_in_maps(**inputs)
    res = bass_utils.run_bass_kernel_spmd(
        nc, in_maps, core_ids=list(range(8)), trace=trace,
        **({"trace_cores": [0]} if trace else {}))
    out = np.stack([res.results[i]["out"] for i in range(8)]).astype(np.float32)
    return out, res


def kernel(**inputs) -> np.ndarray:
    out, _ = run(**inputs)
    return out

